# revision 29
# baseline (speedup 1.0000x reference)
"""Trainium2 Bass kernel for GaussianKernelGCNLayer.

Reference computation (per instance b of 2048 = 8*256):
  wf[b,k,d] = sum_n w[b,n,k] * f[b,n,d]         (n=32 neighbors, k=8 kernels)
  out[b,k,o] = sum_d wf[b,k,d] * CW[k,d,o]      (d=4096, o=512)

Sharding: data-parallel over the 2048 instances -> 256 per core on 8 cores.

Dtypes: the dominant features tensor ships as fp8-e3m4 (globally
scaled into the e3m4 range; the 1/scale is folded into the conv
weights on the host), halving its HBM traffic to 33.5 MB/core. fp8
does NOT speed up the PE itself - measured: LDWEIGHTS runs at 2
elem/cycle/partition for fp8 and fp16 alike (the theoretical 4x fp8
FWL does not materialize; switching phase-2 weights fp16<->fp8e3
changed total time by 0.2 us in 253). All other tensors are fp16; PSUM
accumulation is fp32, so phase-1 sums of e3m4 x fp16 products are
near-exact. Output returns as fp16 and is upcast on the host.
Measured end-to-end rel err 1.328e-2 (threshold 2e-2, HW matches the
numpy quantization sim to 5 digits); fp8 for any second tensor
(conv_weight 1.87e-2, wf-half 1.68e-2, wf-full 1.97e-2) buys no time,
so the margin is kept.

Measured per-core cost model (per-MM ~= N_cols * 0.48 ns [sustained
~2.1 GHz, P0] + ~40 ns serialized weight-change cost; the 40 ns mostly
vanishes when consecutive MMs reuse identical weights - 240 vs 286
ns/MM at N=512 - but every weight tile here is used exactly once):
  Phase 1: 2048 MMs x ~57.5 ns ~= 117 us   (weight-change bound)
  Phase 2:  512 MMs x ~286 ns  ~= 146 us   (stream/FLOP bound)
  DMA: ~70 MB/core at ~340 GB/s ~= 200 us, fully hidden under the PE.
  PSUM-read contention from DVE evacuations: none (no-evac ablation
  times identical). Accumulation is free; start=True costs ~75 ns at
  N=512 (paid once per bank).
The kernel is PE-bound at both phases' structural floors; phase-2's
~125 us of streaming is the pure FLOP floor at 1 col/cycle.

Per-core device algorithm:
  Phase 1: for each group g of 4 instances, stack their (32-neighbor)
    features into a [128, 4096] SBUF tile (contract dim = 4*32 = 128
    partitions) and matmul against a host-prebuilt block-structured
    weight tile [128, 32] whose columns are ordered (k, bi) so the
    psum->SBUF evacuation is a monotonic strided copy (no permute).
    lhsT = feature d-chunk [128, 128] fp8e3 (FWL-eligible: 128
    contiguous columns), rhs = blocked fp16 weights -> psum
    [128(d), 32(k,bi)]: wf TRANSPOSED (d on partitions), exactly the
    layout phase 2 needs.
  Phase 2: for each kernel k: out[b, k*512:+512] = wf_k @ CW_k as 32
    accumulating matmuls over d-chunks; lhsT = wfT[:, c, k, mtile]
    ([128 d, 128 b] contiguous fp16 so FWL engages), rhs = CW chunk
    [128 d, 512 o] sliced from 1 MB host-pretransposed fp16 cw tiles.
"""

import os
import sys

import numpy as np

try:
    import ml_dtypes
except ImportError:  # pragma: no cover
    ml_dtypes = None

for _p in ("/opt/trn_rl_repo",):
    if _p not in sys.path:
        sys.path.insert(0, _p)

NB, NI, NN, DIN = 8, 256, 32, 4096
NK, DKO = 8, 512
NCORES = 8
BL = NB * NI // NCORES  # 256 instances per core
NGRP = BL // 4          # 64 groups of 4 instances
NCH = DIN // 128        # 32 d-chunks
NQ = 4                  # cw DMA quarters per kernel (8 chunks each)
FG = 2                  # instance-groups per f DMA tile (FG/2 MB transfers)
NSG = NGRP // FG        # f DMA tiles
F16 = np.float16
E3M4 = ml_dtypes.float8_e3m4 if ml_dtypes is not None else None
F8_MAX = 15.0           # target absmax after scaling into e3m4 (max 15.5)

_cached_nc = None


def _build(
    repeat=1,
    phases=(1, 2),
    p1_mms=16,
    stag=False,
    hints=False,
    falt=False,
    p2_mts=2,
    p2_dma_all=True,
    p2_same_w=False,
    p2_ncols=DKO,
    p1_gacc=False,
    p2_noacc=False,
    # wf8h: 0 = wfT fp16 (ships; rel err 1.33e-2), 1 = chunks 16-31 in
    # fp8e3 (1.68e-2), 2 = all fp8e3 (1.97e-2). Timing measured IDENTICAL
    # for all three (fp8 FWL is 2x like fp16, not the theoretical 4x), so
    # 0 keeps the maximum accuracy margin for free.
    wf8h=0,
    p1_evac=1,
):
    from contextlib import ExitStack

    import concourse.bass as bass  # noqa: F401
    import concourse.tile as tile
    from concourse import bacc, mybir

    nc = bacc.Bacc(
        "TRN2",
        target_bir_lowering=False,
        debug=False,
        num_devices=NCORES,
    )

    f_d = nc.dram_tensor(
        "fstack", [NSG, 128, FG, DIN], mybir.dt.float8e3, kind="ExternalInput"
    ).ap()
    w_d = nc.dram_tensor(
        "wblk", [128, NGRP, 32], mybir.dt.float16, kind="ExternalInput"
    ).ap()
    cw_d = nc.dram_tensor(
        "cwt", [128, NK, NCH, DKO], mybir.dt.float16, kind="ExternalInput"
    ).ap()
    out_d = nc.dram_tensor(
        "out", [BL, NK * DKO], mybir.dt.float16, kind="ExternalOutput"
    ).ap()

    with ExitStack() as ctx:
        tc = ctx.enter_context(tile.TileContext(nc))
        const_pool = ctx.enter_context(tc.tile_pool(name="const", bufs=1))
        # f tiles (phase 1) and cw tiles (phase 2) have disjoint lifetimes:
        # one shared 6-slot pool gives phase 1 a 6 MB prefetch window with
        # fine-grained slot release (DMA completion-receipt latency hides
        # at >=3-4 transfers in flight) and phase 2 a 6-deep cw pipeline.
        iopool = ctx.enter_context(tc.tile_pool(name="iopool", bufs=7))
        wpool = ctx.enter_context(tc.tile_pool(name="wpool", bufs=1))
        # pt (phase 1) and po (phase 2) are both exactly one PSUM bank with
        # disjoint lifetimes: share all 8 banks for double pipeline depth.
        pspool = ctx.enter_context(tc.tile_pool(name="pspool", bufs=8, space="PSUM"))
        ps1 = pspool
        ps2 = pspool
        opool = ctx.enter_context(tc.tile_pool(name="opool", bufs=2))
        wspool = (
            ctx.enter_context(tc.tile_pool(name="wspool", bufs=2))
            if not p2_dma_all
            else None
        )

        # Persistent transposed wf: [128 (d%128), chunk, k, g*4+bi] fp16.
        # For phase 2, wfT[:, c, k, mt*128:(mt+1)*128] is a single
        # contiguous [128, 128] run -> FWL-eligible weight loads (a 2D
        # [32 g, 4 bi] slice of the same bytes measures ~25 ns/MM slower).
        wf_dts = {
            0: (mybir.dt.float16, mybir.dt.float16),
            1: (mybir.dt.float16, mybir.dt.float8e3),
            2: (mybir.dt.float8e3, mybir.dt.float8e3),
        }[wf8h]
        wfT_h = [
            const_pool.tile(
                [128, NCH // 2, NK, NGRP * 4], wf_dts[h], name=f"wfT{h}"
            )
            for h in range(2)
        ]

        if 1 not in phases:
            # timing-ablation only: give wfT a writer so Tile allocates it
            nc.vector.memset(wfT_h[0][:, :, :, 0:4], 0.0)
            nc.vector.memset(wfT_h[1][:, :, :, 0:4], 0.0)

        if repeat > 1:
            loop_kw = {}
            if stag:
                loop_kw["staggered_reset"] = True
            if hints:
                loop_kw["hint_engines"] = (
                    mybir.EngineType.PE,
                    mybir.EngineType.SP,
                    mybir.EngineType.DVE,
                    mybir.EngineType.Activation,
                )
            ctx.enter_context(tc.For_i(0, repeat, 1, **loop_kw))

        # Blocked neighbour weights for ALL groups: one 512 KB DMA on the
        # ACT ring so the SP ring starts streaming f immediately.
        if 1 in phases:
            wb = wpool.tile([128, NGRP, 32], mybir.dt.float16, name="wb")
            nc.scalar.dma_start(wb[:], w_d[:, :, :])

        # ---- Phase 1: wfT[d, (k,bi)] per instance-group ----
        if 1 in phases:
            for sg in range(NSG):
                fs = iopool.tile([128, FG, DIN], mybir.dt.float8e3, name="fs", tag="io")
                nc.sync.dma_start(fs[:], f_d[sg, :, :, :])
                for g2 in range(FG):
                    g = sg * FG + g2
                    for h in range(2):
                        pt = ps1.tile([128, 16, 8, 4], mybir.dt.float32, name="pt", tag="ps")
                        for cc in range(p1_mms):
                            c = h * 16 + cc
                            nc.tensor.matmul(
                                pt[:, cc, :, :],
                                fs[:, g2, c * 128 : (c + 1) * 128],
                                wb[:, g, :],
                                start=(cc == 0) if p1_gacc else True,
                                stop=(cc == p1_mms - 1) if p1_gacc else True,
                            )
                        # psum [128, 16, k, bi] -> wfT[:, h*16:(h+1)*16, :, g, :]
                        # (same index order on both sides: plain strided copy)
                        if p1_evac == 1:
                            nc.vector.tensor_copy(
                                wfT_h[h][:, 0:p1_mms, :, g * 4 : g * 4 + 4],
                                pt[:, :p1_mms, :, :],
                            )
                        elif p1_evac == 2:
                            nc.scalar.tensor_copy(
                                wfT_h[h][:, 0:p1_mms, :, g * 4 : g * 4 + 4],
                                pt[:, :p1_mms, :, :],
                            )

        # ---- Phase 2: out = wf @ CW, k-outer, both m-tiles per W pass ----
        if 2 in phases:
            wt_shared = None
            if not p2_dma_all:
                wt_shared = wspool.tile(
                    [128, NCH // NQ, DKO], mybir.dt.float16, name="wts"
                )
                nc.sync.dma_start(wt_shared[:], cw_d[:, 0, 0 : NCH // NQ, :])
            for k in range(NK):
                po0 = ps2.tile([128, DKO], mybir.dt.float32, name="po0", tag="ps")
                po1 = ps2.tile([128, DKO], mybir.dt.float32, name="po1", tag="ps")
                pos = (po0, po1)
                for q in range(NQ):
                    if p2_dma_all:
                        wt = iopool.tile(
                            [128, NCH // NQ, DKO],
                            mybir.dt.float16,
                            name="wt",
                            tag="io",
                        )
                        nc.sync.dma_start(
                            wt[:],
                            cw_d[:, k, q * (NCH // NQ) : (q + 1) * (NCH // NQ), :],
                        )
                    else:
                        wt = wt_shared
                    for cc in range(NCH // NQ):
                        c = q * (NCH // NQ) + cc
                        for mt in range(p2_mts):
                            if p2_same_w:
                                lhs = wfT_h[0][:, 0, 0, 0:128]
                            else:
                                lhs = wfT_h[c // 16][
                                    :, c % 16, k, mt * 128 : (mt + 1) * 128
                                ]
                            nc.tensor.matmul(
                                pos[mt][:, 0:p2_ncols],
                                lhs,
                                wt[:, cc, 0:p2_ncols],
                                start=True if p2_noacc else (c == 0),
                                stop=True if p2_noacc else (c == NCH - 1),
                            )
                for mt in range(p2_mts):
                    ot = opool.tile([128, DKO], mybir.dt.float16, name="ot")
                    nc.vector.tensor_copy(ot[:], pos[mt][:])
                    # ACT HWDGE queue: keeps the SP queue free for input DMAs
                    nc.scalar.dma_start(
                        out_d[mt * 128 : (mt + 1) * 128, k * DKO : (k + 1) * DKO],
                        ot[:],
                    )

    nc.compile()
    return nc


def _prep_inputs(neighbourhood_features, neighbourhood_weights, conv_weight):
    f = np.asarray(neighbourhood_features, dtype=np.float32).reshape(
        NB * NI, NN, DIN
    )
    w = np.asarray(neighbourhood_weights, dtype=np.float32).reshape(NB * NI, NN, NK)
    # global scale into the e3m4 range; 1/sf is folded into cwt below
    sf = F8_MAX / max(float(np.abs(f).max()), 1e-30)
    # wf scale: bound |sum_n w*f*sf| <= sum_n w * rowmax|f|*sf per (b,k);
    # folded into wblk (host) so the device psum holds wf pre-scaled into
    # the e3m4 range and the evacuation stays a plain cast-copy.
    fmax = np.abs(f).max(axis=2) * sf                       # [B, NN]
    ub = float(np.einsum("bnk,bn->bk", w, fmax).max())
    swf = F8_MAX / max(ub, 1e-30)
    # cwt[p, k, c, o] = cw[k, c*128+p, o] / (sf*swf)  (shared across cores)
    cw = np.asarray(conv_weight, dtype=np.float32).reshape(NK, NCH, 128, DKO)
    cwt = np.ascontiguousarray(
        cw.transpose(2, 0, 1, 3) * (1.0 / (sf * swf))
    ).astype(F16)
    in_maps = []
    for i in range(NCORES):
        fl = (
            (f[i * BL : (i + 1) * BL] * sf)
            .reshape(NSG, FG, 4 * NN, DIN)
            .transpose(0, 2, 1, 3)
            .astype(E3M4)
        )
        wl = (w[i * BL : (i + 1) * BL] * swf).reshape(NGRP, 4, NN, NK)
        # wblk[bi*32+n, g, k, bi] = wl[g, bi, n, k]; rhs column = k*4+bi
        wblk = np.zeros((128, NGRP, NK, 4), dtype=np.float32)
        for bi in range(4):
            wblk[bi * 32 : (bi + 1) * 32, :, :, bi] = wl[:, bi].transpose(1, 0, 2)
        in_maps.append(
            {
                "fstack": np.ascontiguousarray(fl),
                "wblk": wblk.reshape(128, NGRP, 32).astype(F16),
                "cwt": cwt,
            }
        )
    return in_maps


def _execute(neighbourhood_features, neighbourhood_weights, conv_weight, trace=False):
    global _cached_nc
    if _cached_nc is None:
        _cached_nc = _build()
    nc = _cached_nc
    from concourse import bass_utils

    in_maps = _prep_inputs(
        neighbourhood_features, neighbourhood_weights, conv_weight
    )
    res = bass_utils.run_bass_kernel_spmd(
        nc, in_maps, core_ids=list(range(NCORES)), trace=trace
    )
    outs = [
        np.asarray(res.results[i]["out"]).astype(np.float32) for i in range(NCORES)
    ]
    full = np.concatenate(outs, axis=0)
    return full.reshape(NB, NI, NK * DKO), res


def kernel(neighbourhood_features, neighbourhood_weights, conv_weight):
    out, _ = _execute(
        neighbourhood_features, neighbourhood_weights, conv_weight, trace=False
    )
    return out



# revision 31
# speedup vs baseline: 1.0095x; 1.0095x over previous
"""Trainium2 Bass kernel for GaussianKernelGCNLayer.

Reference computation (per instance b of 2048 = 8*256):
  wf[b,k,d] = sum_n w[b,n,k] * f[b,n,d]         (n=32 neighbors, k=8 kernels)
  out[b,k,o] = sum_d wf[b,k,d] * CW[k,d,o]      (d=4096, o=512)

Sharding: data-parallel over the 2048 instances -> 256 per core on 8 cores.

Dtypes: the dominant features tensor ships as fp8-e3m4 (globally
scaled into the e3m4 range; the 1/scale is folded into the conv
weights on the host), halving its HBM traffic to 33.5 MB/core. fp8
does NOT speed up the PE itself - measured: LDWEIGHTS runs at 2
elem/cycle/partition for fp8 and fp16 alike (the theoretical 4x fp8
FWL does not materialize; switching phase-2 weights fp16<->fp8e3
changed total time by 0.2 us in 253). All other tensors are fp16; PSUM
accumulation is fp32, so phase-1 sums of e3m4 x fp16 products are
near-exact. Output returns as fp16 and is upcast on the host.
Measured end-to-end rel err 1.328e-2 (threshold 2e-2, HW matches the
numpy quantization sim to 5 digits); fp8 for any second tensor
(conv_weight 1.87e-2, wf-half 1.68e-2, wf-full 1.97e-2) buys no time,
so the margin is kept.

Measured per-core cost model (per-MM ~= N_cols * 0.48 ns [sustained
~2.1 GHz, P0] + ~40 ns serialized weight-change cost; the 40 ns mostly
vanishes when consecutive MMs reuse identical weights - 240 vs 286
ns/MM at N=512 - but every weight tile here is used exactly once):
  Phase 1: 2048 MMs x ~57.5 ns ~= 117 us   (weight-change bound)
  Phase 2:  512 MMs x ~286 ns  ~= 146 us   (stream/FLOP bound)
  DMA: ~70 MB/core at ~340 GB/s ~= 200 us, fully hidden under the PE.
  PSUM-read contention from DVE evacuations: none (no-evac ablation
  times identical). Accumulation is free; start=True costs ~75 ns at
  N=512 (paid once per bank).
The kernel is PE-bound at both phases' structural floors; phase-2's
~125 us of streaming is the pure FLOP floor at 1 col/cycle.

Per-core device algorithm:
  Phase 1: for each group g of 4 instances, stack their (32-neighbor)
    features into a [128, 4096] SBUF tile (contract dim = 4*32 = 128
    partitions) and matmul against a host-prebuilt block-structured
    weight tile [128, 32] whose columns are ordered (k, bi) so the
    psum->SBUF evacuation is a monotonic strided copy (no permute).
    lhsT = feature d-chunk [128, 128] fp8e3 (FWL-eligible: 128
    contiguous columns), rhs = blocked fp16 weights -> psum
    [128(d), 32(k,bi)]: wf TRANSPOSED (d on partitions), exactly the
    layout phase 2 needs.
  Phase 2: for each kernel k: out[b, k*512:+512] = wf_k @ CW_k as 32
    accumulating matmuls over d-chunks; lhsT = wfT[:, c, k, mtile]
    ([128 d, 128 b] contiguous fp16 so FWL engages), rhs = CW chunk
    [128 d, 512 o] sliced from 1 MB host-pretransposed fp16 cw tiles.
"""

import os
import sys

import numpy as np

try:
    import ml_dtypes
except ImportError:  # pragma: no cover
    ml_dtypes = None

for _p in ("/opt/trn_rl_repo",):
    if _p not in sys.path:
        sys.path.insert(0, _p)

NB, NI, NN, DIN = 8, 256, 32, 4096
NK, DKO = 8, 512
NCORES = 8
BL = NB * NI // NCORES  # 256 instances per core
NGRP = BL // 4          # 64 groups of 4 instances
NCH = DIN // 128        # 32 d-chunks
NQ = 4                  # cw DMA quarters per kernel (8 chunks each)
FG = 2                  # instance-groups per f DMA tile (FG/2 MB transfers)
NSG = NGRP // FG        # f DMA tiles
F16 = np.float16
E3M4 = ml_dtypes.float8_e3m4 if ml_dtypes is not None else None
F8_MAX = 15.0           # target absmax after scaling into e3m4 (max 15.5)

_cached_nc = None


def _build(
    repeat=1,
    phases=(1, 2),
    p1_mms=16,
    stag=False,
    hints=False,
    falt=False,
    p2_mts=2,
    p2_dma_all=True,
    p2_same_w=False,
    p2_ncols=DKO,
    p1_gacc=False,
    p2_noacc=False,
    # wf8h: 0 = wfT fp16 (ships; rel err 1.33e-2), 1 = chunks 16-31 in
    # fp8e3 (1.68e-2), 2 = all fp8e3 (1.97e-2). Timing measured IDENTICAL
    # for all three (fp8 FWL is 2x like fp16, not the theoretical 4x), so
    # 0 keeps the maximum accuracy margin for free.
    wf8h=0,
    p1_evac=1,
    io_bufs=7,
    nq=NQ,
):
    from contextlib import ExitStack

    import concourse.bass as bass  # noqa: F401
    import concourse.tile as tile
    from concourse import bacc, mybir

    nc = bacc.Bacc(
        "TRN2",
        target_bir_lowering=False,
        debug=False,
        num_devices=NCORES,
    )

    f_d = nc.dram_tensor(
        "fstack", [NSG, 128, FG, DIN], mybir.dt.float8e3, kind="ExternalInput"
    ).ap()
    w_d = nc.dram_tensor(
        "wblk", [128, NGRP, 32], mybir.dt.float16, kind="ExternalInput"
    ).ap()
    cw_d = nc.dram_tensor(
        "cwt", [128, NK, NCH, DKO], mybir.dt.float16, kind="ExternalInput"
    ).ap()
    out_d = nc.dram_tensor(
        "out", [BL, NK * DKO], mybir.dt.float16, kind="ExternalOutput"
    ).ap()

    with ExitStack() as ctx:
        tc = ctx.enter_context(tile.TileContext(nc))
        const_pool = ctx.enter_context(tc.tile_pool(name="const", bufs=1))
        # f tiles (phase 1) and cw tiles (phase 2) have disjoint lifetimes:
        # one shared 6-slot pool gives phase 1 a 6 MB prefetch window with
        # fine-grained slot release (DMA completion-receipt latency hides
        # at >=3-4 transfers in flight) and phase 2 a 6-deep cw pipeline.
        iopool = ctx.enter_context(tc.tile_pool(name="iopool", bufs=io_bufs))
        wpool = ctx.enter_context(tc.tile_pool(name="wpool", bufs=1))
        # pt (phase 1) and po (phase 2) are both exactly one PSUM bank with
        # disjoint lifetimes: share all 8 banks for double pipeline depth.
        pspool = ctx.enter_context(tc.tile_pool(name="pspool", bufs=8, space="PSUM"))
        ps1 = pspool
        ps2 = pspool
        opool = ctx.enter_context(tc.tile_pool(name="opool", bufs=2))
        wspool = (
            ctx.enter_context(tc.tile_pool(name="wspool", bufs=2))
            if not p2_dma_all
            else None
        )

        # Persistent transposed wf: [128 (d%128), chunk, k, g*4+bi] fp16.
        # For phase 2, wfT[:, c, k, mt*128:(mt+1)*128] is a single
        # contiguous [128, 128] run -> FWL-eligible weight loads (a 2D
        # [32 g, 4 bi] slice of the same bytes measures ~25 ns/MM slower).
        wf_dts = {
            0: (mybir.dt.float16, mybir.dt.float16),
            1: (mybir.dt.float16, mybir.dt.float8e3),
            2: (mybir.dt.float8e3, mybir.dt.float8e3),
        }[wf8h]
        wfT_h = [
            const_pool.tile(
                [128, NCH // 2, NK, NGRP * 4], wf_dts[h], name=f"wfT{h}"
            )
            for h in range(2)
        ]

        if 1 not in phases:
            # timing-ablation only: give wfT a writer so Tile allocates it
            nc.vector.memset(wfT_h[0][:, :, :, 0:4], 0.0)
            nc.vector.memset(wfT_h[1][:, :, :, 0:4], 0.0)

        if repeat > 1:
            loop_kw = {}
            if stag:
                loop_kw["staggered_reset"] = True
            if hints:
                loop_kw["hint_engines"] = (
                    mybir.EngineType.PE,
                    mybir.EngineType.SP,
                    mybir.EngineType.DVE,
                    mybir.EngineType.Activation,
                )
            ctx.enter_context(tc.For_i(0, repeat, 1, **loop_kw))

        # Blocked neighbour weights for ALL groups: one 512 KB DMA on the
        # ACT ring so the SP ring starts streaming f immediately.
        if 1 in phases:
            wb = wpool.tile([128, NGRP, 32], mybir.dt.float16, name="wb")
            nc.scalar.dma_start(wb[:], w_d[:, :, :])

        # ---- Phase 1: wfT[d, (k,bi)] per instance-group ----
        if 1 in phases:
            for sg in range(NSG):
                fs = iopool.tile([128, FG, DIN], mybir.dt.float8e3, name="fs", tag="io")
                nc.sync.dma_start(fs[:], f_d[sg, :, :, :])
                for g2 in range(FG):
                    g = sg * FG + g2
                    for h in range(2):
                        pt = ps1.tile([128, 16, 8, 4], mybir.dt.float32, name="pt", tag="ps")
                        for cc in range(p1_mms):
                            c = h * 16 + cc
                            nc.tensor.matmul(
                                pt[:, cc, :, :],
                                fs[:, g2, c * 128 : (c + 1) * 128],
                                wb[:, g, :],
                                start=(cc == 0) if p1_gacc else True,
                                stop=(cc == p1_mms - 1) if p1_gacc else True,
                            )
                        # psum [128, 16, k, bi] -> wfT[:, h*16:(h+1)*16, :, g, :]
                        # (same index order on both sides: plain strided copy)
                        if p1_evac:
                            nc.vector.tensor_copy(
                                wfT_h[h][:, 0:p1_mms, :, g * 4 : g * 4 + 4],
                                pt[:, :p1_mms, :, :],
                            )

        # ---- Phase 2: out = wf @ CW, k-outer, both m-tiles per W pass ----
        if 2 in phases:
            wt_shared = None
            if not p2_dma_all:
                wt_shared = wspool.tile(
                    [128, NCH // nq, DKO], mybir.dt.float16, name="wts"
                )
                nc.sync.dma_start(wt_shared[:], cw_d[:, 0, 0 : NCH // nq, :])
            for k in range(NK):
                po0 = ps2.tile([128, DKO], mybir.dt.float32, name="po0", tag="ps")
                po1 = ps2.tile([128, DKO], mybir.dt.float32, name="po1", tag="ps")
                pos = (po0, po1)
                for q in range(nq):
                    if p2_dma_all:
                        wt = iopool.tile(
                            [128, NCH // nq, DKO],
                            mybir.dt.float16,
                            name="wt",
                            tag="io",
                        )
                        nc.sync.dma_start(
                            wt[:],
                            cw_d[:, k, q * (NCH // nq) : (q + 1) * (NCH // nq), :],
                        )
                    else:
                        wt = wt_shared
                    for cc in range(NCH // nq):
                        c = q * (NCH // nq) + cc
                        for mt in range(p2_mts):
                            if p2_same_w:
                                lhs = wfT_h[0][:, 0, 0, 0:128]
                            else:
                                lhs = wfT_h[c // 16][
                                    :, c % 16, k, mt * 128 : (mt + 1) * 128
                                ]
                            nc.tensor.matmul(
                                pos[mt][:, 0:p2_ncols],
                                lhs,
                                wt[:, cc, 0:p2_ncols],
                                start=True if p2_noacc else (c == 0),
                                stop=True if p2_noacc else (c == NCH - 1),
                            )
                for mt in range(p2_mts):
                    ot = opool.tile([128, DKO], mybir.dt.float16, name="ot")
                    nc.vector.tensor_copy(ot[:], pos[mt][:])
                    # ACT HWDGE queue: keeps the SP queue free for input DMAs
                    nc.scalar.dma_start(
                        out_d[mt * 128 : (mt + 1) * 128, k * DKO : (k + 1) * DKO],
                        ot[:],
                    )

    nc.compile()
    return nc


def _prep_inputs(neighbourhood_features, neighbourhood_weights, conv_weight):
    f = np.asarray(neighbourhood_features, dtype=np.float32).reshape(
        NB * NI, NN, DIN
    )
    w = np.asarray(neighbourhood_weights, dtype=np.float32).reshape(NB * NI, NN, NK)
    # global scale into the e3m4 range; 1/sf is folded into cwt below
    sf = F8_MAX / max(float(np.abs(f).max()), 1e-30)
    # wf scale: bound |sum_n w*f*sf| <= sum_n w * rowmax|f|*sf per (b,k);
    # folded into wblk (host) so the device psum holds wf pre-scaled into
    # the e3m4 range and the evacuation stays a plain cast-copy.
    fmax = np.abs(f).max(axis=2) * sf                       # [B, NN]
    ub = float(np.einsum("bnk,bn->bk", w, fmax).max())
    swf = F8_MAX / max(ub, 1e-30)
    # cwt[p, k, c, o] = cw[k, c*128+p, o] / (sf*swf)  (shared across cores)
    cw = np.asarray(conv_weight, dtype=np.float32).reshape(NK, NCH, 128, DKO)
    cwt = np.ascontiguousarray(
        cw.transpose(2, 0, 1, 3) * (1.0 / (sf * swf))
    ).astype(F16)
    in_maps = []
    for i in range(NCORES):
        fl = (
            (f[i * BL : (i + 1) * BL] * sf)
            .reshape(NSG, FG, 4 * NN, DIN)
            .transpose(0, 2, 1, 3)
            .astype(E3M4)
        )
        wl = (w[i * BL : (i + 1) * BL] * swf).reshape(NGRP, 4, NN, NK)
        # wblk[bi*32+n, g, k, bi] = wl[g, bi, n, k]; rhs column = k*4+bi
        wblk = np.zeros((128, NGRP, NK, 4), dtype=np.float32)
        for bi in range(4):
            wblk[bi * 32 : (bi + 1) * 32, :, :, bi] = wl[:, bi].transpose(1, 0, 2)
        in_maps.append(
            {
                "fstack": np.ascontiguousarray(fl),
                "wblk": wblk.reshape(128, NGRP, 32).astype(F16),
                "cwt": cwt,
            }
        )
    return in_maps


def _execute(neighbourhood_features, neighbourhood_weights, conv_weight, trace=False):
    global _cached_nc
    if _cached_nc is None:
        _cached_nc = _build()
    nc = _cached_nc
    from concourse import bass_utils

    in_maps = _prep_inputs(
        neighbourhood_features, neighbourhood_weights, conv_weight
    )
    res = bass_utils.run_bass_kernel_spmd(
        nc, in_maps, core_ids=list(range(NCORES)), trace=trace
    )
    outs = [
        np.asarray(res.results[i]["out"]).astype(np.float32) for i in range(NCORES)
    ]
    full = np.concatenate(outs, axis=0)
    return full.reshape(NB, NI, NK * DKO), res


def kernel(neighbourhood_features, neighbourhood_weights, conv_weight):
    out, _ = _execute(
        neighbourhood_features, neighbourhood_weights, conv_weight, trace=False
    )
    return out



# revision 32
# speedup vs baseline: 1.0101x; 1.0006x over previous
"""Trainium2 Bass kernel for GaussianKernelGCNLayer.

Reference computation (per instance b of 2048 = 8*256):
  wf[b,k,d] = sum_n w[b,n,k] * f[b,n,d]         (n=32 neighbors, k=8 kernels)
  out[b,k,o] = sum_d wf[b,k,d] * CW[k,d,o]      (d=4096, o=512)

Sharding: data-parallel over the 2048 instances -> 256 per core on 8 cores.

Dtypes: the dominant features tensor ships as fp8-e3m4 (globally
scaled into the e3m4 range; the 1/scale is folded into the conv
weights on the host), halving its HBM traffic to 33.5 MB/core. fp8
does NOT speed up the PE itself - measured: LDWEIGHTS runs at 2
elem/cycle/partition for fp8 and fp16 alike (the theoretical 4x fp8
FWL does not materialize; switching phase-2 weights fp16<->fp8e3
changed total time by 0.2 us in 253). All other tensors are fp16; PSUM
accumulation is fp32, so phase-1 sums of e3m4 x fp16 products are
near-exact. Output returns as fp16 and is upcast on the host.
Measured end-to-end rel err 1.328e-2 (threshold 2e-2, HW matches the
numpy quantization sim to 5 digits); fp8 for any second tensor
(conv_weight 1.87e-2, wf-half 1.68e-2, wf-full 1.97e-2) buys no time,
so the margin is kept.

Measured per-core cost model (per-MM ~= N_cols * 0.48 ns [sustained
~2.1 GHz, P0] + ~40 ns serialized weight-change cost; the 40 ns mostly
vanishes when consecutive MMs reuse identical weights - 240 vs 286
ns/MM at N=512 - but every weight tile here is used exactly once):
  Phase 1: 2048 MMs x ~57.5 ns ~= 117 us   (weight-change bound)
  Phase 2:  512 MMs x ~286 ns  ~= 146 us   (stream/FLOP bound)
  DMA: ~70 MB/core at ~340 GB/s ~= 200 us, fully hidden under the PE.
  PSUM-read contention from DVE evacuations: none (no-evac ablation
  times identical). Accumulation is free; start=True costs ~75 ns at
  N=512 (paid once per bank).
The kernel is PE-bound at both phases' structural floors; phase-2's
~125 us of streaming is the pure FLOP floor at 1 col/cycle.

Per-core device algorithm:
  Phase 1: for each group g of 4 instances, stack their (32-neighbor)
    features into a [128, 4096] SBUF tile (contract dim = 4*32 = 128
    partitions) and matmul against a host-prebuilt block-structured
    weight tile [128, 32] whose columns are ordered (k, bi) so the
    psum->SBUF evacuation is a monotonic strided copy (no permute).
    lhsT = feature d-chunk [128, 128] fp8e3 (FWL-eligible: 128
    contiguous columns), rhs = blocked fp16 weights -> psum
    [128(d), 32(k,bi)]: wf TRANSPOSED (d on partitions), exactly the
    layout phase 2 needs.
  Phase 2: for each kernel k: out[b, k*512:+512] = wf_k @ CW_k as 32
    accumulating matmuls over d-chunks; lhsT = wfT[:, c, k, mtile]
    ([128 d, 128 b] contiguous fp16 so FWL engages), rhs = CW chunk
    [128 d, 512 o] sliced from 1 MB host-pretransposed fp16 cw tiles.
"""

import os
import sys

import numpy as np

try:
    import ml_dtypes
except ImportError:  # pragma: no cover
    ml_dtypes = None

for _p in ("/opt/trn_rl_repo",):
    if _p not in sys.path:
        sys.path.insert(0, _p)

NB, NI, NN, DIN = 8, 256, 32, 4096
NK, DKO = 8, 512
NCORES = 8
BL = NB * NI // NCORES  # 256 instances per core
NGRP = BL // 4          # 64 groups of 4 instances
NCH = DIN // 128        # 32 d-chunks
NQ = 4                  # cw DMA quarters per kernel (8 chunks each)
FG = 2                  # instance-groups per f DMA tile (FG/2 MB transfers)
NSG = NGRP // FG        # f DMA tiles
F16 = np.float16
E3M4 = ml_dtypes.float8_e3m4 if ml_dtypes is not None else None
F8_MAX = 15.0           # target absmax after scaling into e3m4 (max 15.5)

_cached_nc = None


def _build(
    repeat=1,
    phases=(1, 2),
    p1_mms=16,
    stag=False,
    hints=False,
    falt=False,
    p2_mts=2,
    p2_dma_all=True,
    p2_same_w=False,
    p2_ncols=DKO,
    p1_gacc=False,
    p2_noacc=False,
    # wf8h: 0 = wfT fp16 (ships; rel err 1.33e-2), 1 = chunks 16-31 in
    # fp8e3 (1.68e-2), 2 = all fp8e3 (1.97e-2). Timing measured IDENTICAL
    # for all three (fp8 FWL is 2x like fp16, not the theoretical 4x), so
    # 0 keeps the maximum accuracy margin for free.
    wf8h=0,
    p1_evac=1,
    io_bufs=7,
    nq=NQ,
    p2_banks=0,
):
    from contextlib import ExitStack

    import concourse.bass as bass  # noqa: F401
    import concourse.tile as tile
    from concourse import bacc, mybir

    nc = bacc.Bacc(
        "TRN2",
        target_bir_lowering=False,
        debug=False,
        num_devices=NCORES,
    )

    f_d = nc.dram_tensor(
        "fstack", [NSG, 128, FG, DIN], mybir.dt.float8e3, kind="ExternalInput"
    ).ap()
    w_d = nc.dram_tensor(
        "wblk", [128, NGRP, 32], mybir.dt.float16, kind="ExternalInput"
    ).ap()
    cw_d = nc.dram_tensor(
        "cwt", [128, NK, NCH, DKO], mybir.dt.float16, kind="ExternalInput"
    ).ap()
    out_d = nc.dram_tensor(
        "out", [BL, NK * DKO], mybir.dt.float16, kind="ExternalOutput"
    ).ap()

    with ExitStack() as ctx:
        tc = ctx.enter_context(tile.TileContext(nc))
        const_pool = ctx.enter_context(tc.tile_pool(name="const", bufs=1))
        # f tiles (phase 1) and cw tiles (phase 2) have disjoint lifetimes:
        # one shared 6-slot pool gives phase 1 a 6 MB prefetch window with
        # fine-grained slot release (DMA completion-receipt latency hides
        # at >=3-4 transfers in flight) and phase 2 a 6-deep cw pipeline.
        iopool = ctx.enter_context(tc.tile_pool(name="iopool", bufs=io_bufs))
        wpool = ctx.enter_context(tc.tile_pool(name="wpool", bufs=1))
        # pt (phase 1) and po (phase 2) are both exactly one PSUM bank with
        # disjoint lifetimes: share all 8 banks for double pipeline depth.
        pspool = ctx.enter_context(
            tc.tile_pool(name="pspool", bufs=(8 - p2_banks), space="PSUM")
        )
        ps1 = pspool
        ps2 = (
            ctx.enter_context(tc.tile_pool(name="ps2pool", bufs=p2_banks, space="PSUM"))
            if p2_banks
            else pspool
        )
        opool = ctx.enter_context(tc.tile_pool(name="opool", bufs=2))
        wspool = (
            ctx.enter_context(tc.tile_pool(name="wspool", bufs=2))
            if not p2_dma_all
            else None
        )

        # Persistent transposed wf: [128 (d%128), chunk, k, g*4+bi] fp16.
        # For phase 2, wfT[:, c, k, mt*128:(mt+1)*128] is a single
        # contiguous [128, 128] run -> FWL-eligible weight loads (a 2D
        # [32 g, 4 bi] slice of the same bytes measures ~25 ns/MM slower).
        wf_dts = {
            0: (mybir.dt.float16, mybir.dt.float16),
            1: (mybir.dt.float16, mybir.dt.float8e3),
            2: (mybir.dt.float8e3, mybir.dt.float8e3),
        }[wf8h]
        wfT_h = [
            const_pool.tile(
                [128, NCH // 2, NK, NGRP * 4], wf_dts[h], name=f"wfT{h}"
            )
            for h in range(2)
        ]

        if 1 not in phases:
            # timing-ablation only: give wfT a writer so Tile allocates it
            nc.vector.memset(wfT_h[0][:, :, :, 0:4], 0.0)
            nc.vector.memset(wfT_h[1][:, :, :, 0:4], 0.0)

        if repeat > 1:
            loop_kw = {}
            if stag:
                loop_kw["staggered_reset"] = True
            if hints:
                loop_kw["hint_engines"] = (
                    mybir.EngineType.PE,
                    mybir.EngineType.SP,
                    mybir.EngineType.DVE,
                    mybir.EngineType.Activation,
                )
            ctx.enter_context(tc.For_i(0, repeat, 1, **loop_kw))

        # Blocked neighbour weights for ALL groups: one 512 KB DMA on the
        # ACT ring so the SP ring starts streaming f immediately.
        if 1 in phases:
            wb = wpool.tile([128, NGRP, 32], mybir.dt.float16, name="wb")
            nc.scalar.dma_start(wb[:], w_d[:, :, :])

        # ---- Phase 1: wfT[d, (k,bi)] per instance-group ----
        if 1 in phases:
            for sg in range(NSG):
                fs = iopool.tile([128, FG, DIN], mybir.dt.float8e3, name="fs", tag="io")
                nc.sync.dma_start(fs[:], f_d[sg, :, :, :])
                for g2 in range(FG):
                    g = sg * FG + g2
                    for h in range(2):
                        pt = ps1.tile([128, 16, 8, 4], mybir.dt.float32, name="pt", tag="ps")
                        for cc in range(p1_mms):
                            c = h * 16 + cc
                            nc.tensor.matmul(
                                pt[:, cc, :, :],
                                fs[:, g2, c * 128 : (c + 1) * 128],
                                wb[:, g, :],
                                start=(cc == 0) if p1_gacc else True,
                                stop=(cc == p1_mms - 1) if p1_gacc else True,
                            )
                        # psum [128, 16, k, bi] -> wfT[:, h*16:(h+1)*16, :, g, :]
                        # (same index order on both sides: plain strided copy)
                        if p1_evac:
                            nc.vector.tensor_copy(
                                wfT_h[h][:, 0:p1_mms, :, g * 4 : g * 4 + 4],
                                pt[:, :p1_mms, :, :],
                            )

        # ---- Phase 2: out = wf @ CW, k-outer, both m-tiles per W pass ----
        if 2 in phases:
            wt_shared = None
            if not p2_dma_all:
                wt_shared = wspool.tile(
                    [128, NCH // nq, DKO], mybir.dt.float16, name="wts"
                )
                nc.sync.dma_start(wt_shared[:], cw_d[:, 0, 0 : NCH // nq, :])
            for k in range(NK):
                po0 = ps2.tile([128, DKO], mybir.dt.float32, name="po0", tag="ps")
                po1 = ps2.tile([128, DKO], mybir.dt.float32, name="po1", tag="ps")
                pos = (po0, po1)
                for q in range(nq):
                    if p2_dma_all:
                        wt = iopool.tile(
                            [128, NCH // nq, DKO],
                            mybir.dt.float16,
                            name="wt",
                            tag="io",
                        )
                        nc.sync.dma_start(
                            wt[:],
                            cw_d[:, k, q * (NCH // nq) : (q + 1) * (NCH // nq), :],
                        )
                    else:
                        wt = wt_shared
                    for cc in range(NCH // nq):
                        c = q * (NCH // nq) + cc
                        for mt in range(p2_mts):
                            if p2_same_w:
                                lhs = wfT_h[0][:, 0, 0, 0:128]
                            else:
                                lhs = wfT_h[c // 16][
                                    :, c % 16, k, mt * 128 : (mt + 1) * 128
                                ]
                            nc.tensor.matmul(
                                pos[mt][:, 0:p2_ncols],
                                lhs,
                                wt[:, cc, 0:p2_ncols],
                                start=True if p2_noacc else (c == 0),
                                stop=True if p2_noacc else (c == NCH - 1),
                            )
                for mt in range(p2_mts):
                    ot = opool.tile([128, DKO], mybir.dt.float16, name="ot")
                    nc.vector.tensor_copy(ot[:], pos[mt][:])
                    # ACT HWDGE queue: keeps the SP queue free for input DMAs
                    nc.scalar.dma_start(
                        out_d[mt * 128 : (mt + 1) * 128, k * DKO : (k + 1) * DKO],
                        ot[:],
                    )

    nc.compile()
    return nc


def _prep_inputs(neighbourhood_features, neighbourhood_weights, conv_weight):
    f = np.asarray(neighbourhood_features, dtype=np.float32).reshape(
        NB * NI, NN, DIN
    )
    w = np.asarray(neighbourhood_weights, dtype=np.float32).reshape(NB * NI, NN, NK)
    # global scale into the e3m4 range; 1/sf is folded into cwt below
    sf = F8_MAX / max(float(np.abs(f).max()), 1e-30)
    # wf scale: bound |sum_n w*f*sf| <= sum_n w * rowmax|f|*sf per (b,k);
    # folded into wblk (host) so the device psum holds wf pre-scaled into
    # the e3m4 range and the evacuation stays a plain cast-copy.
    fmax = np.abs(f).max(axis=2) * sf                       # [B, NN]
    ub = float(np.einsum("bnk,bn->bk", w, fmax).max())
    swf = F8_MAX / max(ub, 1e-30)
    # cwt[p, k, c, o] = cw[k, c*128+p, o] / (sf*swf)  (shared across cores)
    cw = np.asarray(conv_weight, dtype=np.float32).reshape(NK, NCH, 128, DKO)
    cwt = np.ascontiguousarray(
        cw.transpose(2, 0, 1, 3) * (1.0 / (sf * swf))
    ).astype(F16)
    in_maps = []
    for i in range(NCORES):
        fl = (
            (f[i * BL : (i + 1) * BL] * sf)
            .reshape(NSG, FG, 4 * NN, DIN)
            .transpose(0, 2, 1, 3)
            .astype(E3M4)
        )
        wl = (w[i * BL : (i + 1) * BL] * swf).reshape(NGRP, 4, NN, NK)
        # wblk[bi*32+n, g, k, bi] = wl[g, bi, n, k]; rhs column = k*4+bi
        wblk = np.zeros((128, NGRP, NK, 4), dtype=np.float32)
        for bi in range(4):
            wblk[bi * 32 : (bi + 1) * 32, :, :, bi] = wl[:, bi].transpose(1, 0, 2)
        in_maps.append(
            {
                "fstack": np.ascontiguousarray(fl),
                "wblk": wblk.reshape(128, NGRP, 32).astype(F16),
                "cwt": cwt,
            }
        )
    return in_maps


def _execute(neighbourhood_features, neighbourhood_weights, conv_weight, trace=False):
    global _cached_nc
    if _cached_nc is None:
        _cached_nc = _build()
    nc = _cached_nc
    from concourse import bass_utils

    in_maps = _prep_inputs(
        neighbourhood_features, neighbourhood_weights, conv_weight
    )
    res = bass_utils.run_bass_kernel_spmd(
        nc, in_maps, core_ids=list(range(NCORES)), trace=trace
    )
    outs = [
        np.asarray(res.results[i]["out"]).astype(np.float32) for i in range(NCORES)
    ]
    full = np.concatenate(outs, axis=0)
    return full.reshape(NB, NI, NK * DKO), res


def kernel(neighbourhood_features, neighbourhood_weights, conv_weight):
    out, _ = _execute(
        neighbourhood_features, neighbourhood_weights, conv_weight, trace=False
    )
    return out

